# revision 36
# baseline (speedup 1.0000x reference)
"""Trainium2 Bass kernel for nn_CMAttention (Infini-attention with compressive memory).

Sharding: 8 cores = 2 (batch) x 4 (head-groups of 4 heads). Each core computes the
full packed sequence [audio(2048); x(2048)] for its batch and its 4 heads:
QKV projection, multi-head rmsnorm, RoPE, segment-local causal attention plus the
linear compressive memory, and writes its [2048, 256] slices of out_x / out_a.

fp16 data path: inputs (activations, weights, rope tables) are cast to fp16 on the
host; all SBUF intermediates are fp16 (PSUM accumulation stays fp32); outputs are
DMA'd as fp16 and upcast on the host. Work is spread across PE/ACT/DVE/Pool.

Self-contained: only numpy + the environment's concourse libraries.
"""
from contextlib import ExitStack

import numpy as np

# problem dims (hardcoded per contract)
DIM = 1024
HEADS = 16
DH = 64
SEG = 1024
B = 2
NA = 2048
NX = 2048
NTOT = NA + NX
N_CORES = 8
HL = 4              # heads per core
NSEG = NTOT // SEG  # 4
ST = SEG // 128     # 8 seq tiles per segment
KC = DIM // 128     # 8 contraction chunks

_PERM = np.concatenate([np.arange(0, DH, 2), np.arange(1, DH, 2)])  # evens, odds
_LN8 = float(np.log(8.0))


def build_program():
    import concourse.tile as tile
    import concourse.mybir as mybir
    from concourse import bacc
    from concourse.masks import make_identity, make_upper_triangular

    dt = mybir.dt
    AF = mybir.ActivationFunctionType
    ALU = mybir.AluOpType
    AX = mybir.AxisListType

    import concourse.mybir as _mb
    import bass_rust as _br
    from concourse.hw_specs import get_activation_tables as _gat

    class _Bacc(bacc.Bacc):
        def insert_act_table_loads(self):
            has_act = any(isinstance(i, _mb.InstActivation)
                          for b_ in self.main_func.blocks
                          for i in b_.instructions)
            if not has_act:
                return
            keep = "natural_log_exp_and_others"
            tables = [(nm, fns if nm == keep else set())
                      for nm, fns in _gat(self.m.arch).items()]
            _br.insert_act_table_loads(self, tables)

    nc = _Bacc("TRN2", target_bir_lowering=False, debug=False,
               num_devices=N_CORES, name="cmattn")

    # ---- DRAM I/O (per core), fp16 ----
    at_d = nc.dram_tensor("at", (DIM, NA), dt.float16, kind="ExternalInput")
    xt_d = nc.dram_tensor("xt", (DIM, NX), dt.float16, kind="ExternalInput")
    wa_d = nc.dram_tensor("wa", (DIM, 3 * HL * DH), dt.float16, kind="ExternalInput")
    wx_d = nc.dram_tensor("wx", (DIM, 3 * HL * DH), dt.float16, kind="ExternalInput")
    cos_d = nc.dram_tensor("cos2", (128, NSEG * ST, DH), dt.float16, kind="ExternalInput")
    sin_d = nc.dram_tensor("sin2", (128, NSEG * ST, DH), dt.float16, kind="ExternalInput")
    # gvec rows: [sigmoid(gate), 1/(1-sigmoid(gate))]
    g_d = nc.dram_tensor("gvec", (2, HL), dt.float32, kind="ExternalInput")
    oa_d = nc.dram_tensor("out_a", (NA, HL * DH), dt.float16, kind="ExternalOutput")
    ox_d = nc.dram_tensor("out_x", (NX, HL * DH), dt.float16, kind="ExternalOutput")

    oa_r = oa_d.ap().rearrange("(t p) c -> p t c", p=128)   # [128, 16, 256]
    ox_r = ox_d.ap().rearrange("(t p) c -> p t c", p=128)

    with tile.TileContext(nc) as tc, ExitStack() as ctx:
        cpool = ctx.enter_context(tc.tile_pool(name="const", bufs=1))
        wpool = ctx.enter_context(tc.tile_pool(name="wpool", bufs=1))
        spool2 = ctx.enter_context(tc.tile_pool(name="spool2", bufs=2))
        ppool = ctx.enter_context(tc.tile_pool(name="ppool", bufs=1))
        tpool = ctx.enter_context(tc.tile_pool(name="tpool", bufs=2))
        bpool = ctx.enter_context(tc.tile_pool(name="bpool", bufs=1))
        psA = ctx.enter_context(tc.tile_pool(name="psA", bufs=2, space="PSUM"))
        psB = ctx.enter_context(tc.tile_pool(name="psB", bufs=4, space="PSUM"))

        # ---- constants ----
        ident_f = cpool.tile([128, 128], dt.float32, tag="identf")
        make_identity(nc, ident_f[:])
        ident = cpool.tile([128, 128], dt.float16, tag="ident")
        nc.vector.tensor_copy(ident[:], ident_f[:])
        tri01 = cpool.tile([128, 128], dt.float16, tag="tri")
        make_upper_triangular(nc, tri01[:], val=1.0, diag=True)  # 1 where k<=q
        g2_sb = cpool.tile([128, 2, HL], dt.float32, tag="g")
        nc.sync.dma_start(g2_sb[:], g_d.ap()[None, :, :].to_broadcast((128, 2, HL)))
        g_sb = g2_sb[:, 0, :]      # sigmoid(gate)
        iomg16 = cpool.tile([128, HL], dt.float16, tag="iomg")
        nc.vector.tensor_copy(iomg16[:], g2_sb[:, 1, :])  # 1/(1-g), fp16
        # M layout: col 0 = z, cols 1..64 = M values (both halves duplicated)
        M_sb = cpool.tile([128, HL, DH + 1], dt.float32, tag="M")
        nc.vector.memset(M_sb[:], 0.0)
        # M16g: fp16 copy of M with the value part pre-scaled by sigmoid(gate)
        M16g = cpool.tile([128, HL, DH + 1], dt.float16, tag="M16")
        ln8_sb = cpool.tile([128, 1], dt.float32, tag="ln8")
        nc.vector.memset(ln8_sb[:], _LN8)

        # weights + rope tables: load everything upfront, on the Pool DMA
        # queue so they don't serialize behind the seg-0 activations on SP
        wa_sb = cpool.tile([128, KC, 3 * HL * DH], dt.float16, tag="wa")
        wx_sb = cpool.tile([128, KC, 3 * HL * DH], dt.float16, tag="wx")
        nc.gpsimd.dma_start(wa_sb[:], wa_d.ap().rearrange("(k p) c -> p k c", p=128))
        nc.gpsimd.dma_start(wx_sb[:], wx_d.ap().rearrange("(k p) c -> p k c", p=128))
        cos_sb = cpool.tile([128, NSEG * ST, DH], dt.float16, tag="cos")
        sin_sb = cpool.tile([128, NSEG * ST, DH], dt.float16, tag="sin")
        nc.gpsimd.dma_start(cos_sb[:], cos_d.ap())
        nc.gpsimd.dma_start(sin_sb[:], sin_d.ap())

        for seg in range(NSEG):
            src_d = at_d if seg < 2 else xt_d
            w_sb = wa_sb if seg < 2 else wx_sb
            col0 = (seg % 2) * SEG

            xt_sb = spool2.tile([128, KC, SEG], dt.float16, tag="xt")
            src_r = src_d.ap().rearrange("(k p) c -> p k c", p=128)
            nc.sync.dma_start(
                xt_sb[:, :, 0:SEG // 2],
                src_r[:, :, col0:col0 + SEG // 2])
            nc.sync.dma_start(
                xt_sb[:, :, SEG // 2:SEG],
                src_r[:, :, col0 + SEG // 2:col0 + SEG])

            # qkT rows: [q pair0, q pair1, k pair0, k pair1]
            qkT = spool2.tile([128, 4, SEG], dt.float16, tag="qkT")
            sqT = spool2.tile([128, 2, SEG], dt.float16, tag="sqT")
            qkn = spool2.tile([128, ST, 2 * HL, DH], dt.float16, tag="qkn")
            sk_all = spool2.tile([128, ST, HL, 2, DH], dt.float16, tag="skal")
            # v16 cols: 0 = 1.0 (z accumulator), 1..64 = v, 65 = 1/(1-g)
            v16 = spool2.tile([128, ST, HL, DH + 2], dt.float16, tag="v16")
            nc.gpsimd.memset(v16[:, :, :, 0], 1.0)
            nc.gpsimd.tensor_copy(
                v16[:, :, :, DH + 1],
                iomg16[:, None, :].to_broadcast((128, ST, HL)))
            outst = spool2.tile([128, ST, HL, DH], dt.float16, tag="outst")

            # ---- phase 1: projection + norm + rope + transposes, st-pairs ----
            for sp in range(ST // 2):
                # projection + psum->sbuf copies for both tiles of the pair
                qk16 = tpool.tile([128, 2, 2 * HL * DH], dt.float16, tag="qk16")
                for u in range(2):
                    st = 2 * sp + u
                    pj_ps = psA.tile([128, 4, 256], dt.float32, tag="big")
                    qk_ps = pj_ps[:].rearrange("p a b -> p (a b)")[:, 0:512]
                    v_ps = pj_ps[:].rearrange("p a b -> p (a b)")[:, 512:768]
                    for kc in range(KC):
                        lhsT = xt_sb[:, kc, st * 128:(st + 1) * 128]
                        nc.tensor.matmul(qk_ps, lhsT=lhsT,
                                         rhs=w_sb[:, kc, 0:512],
                                         start=(kc == 0), stop=(kc == KC - 1))
                        nc.tensor.matmul(v_ps, lhsT=lhsT,
                                         rhs=w_sb[:, kc, 512:768],
                                         start=(kc == 0), stop=(kc == KC - 1))
                    nc.vector.tensor_copy(
                        v16[:, st, :, 1:DH + 1],
                        v_ps.rearrange("p (h d) -> p h d", h=HL))
                    nc.scalar.copy(qk16[:, u, :], qk_ps)

                # batched norm chain over the pair: square (Pool), reduce (DVE),
                # ln/exp (ACT), rsc expand (Pool)
                UH = 2 * 2 * HL  # (u h) = 16
                qkv16 = qk16[:].rearrange("p u (h d) -> p (u h) d", d=DH)
                tsq = bpool.tile([128, UH, DH], dt.float16, tag="tsq")
                nc.gpsimd.tensor_tensor(tsq[:], qkv16, qkv16, ALU.mult)
                ss = tpool.tile([128, UH], dt.float16, tag="ss")
                with nc.allow_low_precision(reason="fp16 sum of 64 squares, ~0.5% err"):
                    nc.vector.tensor_reduce(ss[:], tsq[:], axis=AX.X, op=ALU.add)
                # r = 8 / sqrt(ss) = exp(-0.5*ln(ss) + ln8)
                lnv = tpool.tile([128, UH], dt.float32, tag="lnv")
                nc.scalar.activation(lnv[:], ss[:], AF.Ln)
                rsc = tpool.tile([128, UH], dt.float16, tag="rsc")
                nc.scalar.activation(rsc[:], lnv[:], AF.Exp, scale=-0.5,
                                     bias=ln8_sb[:, 0:1])
                rscx = tpool.tile([128, UH, DH], dt.float16, tag="rscx")
                nc.gpsimd.tensor_copy(
                    rscx[:].rearrange("p h d -> p d h"),
                    rsc[:, None, :].to_broadcast((128, DH, UH)))

                gst = seg * ST + 2 * sp
                cosb2 = cos_sb[:, gst:gst + 2, None, :].to_broadcast(
                    (128, 2, 2 * HL, DH))
                sinb2 = sin_sb[:, gst:gst + 2, None, :].to_broadcast(
                    (128, 2, 2 * HL, DH))

                nrm = bpool.tile([128, UH, DH], dt.float16, tag="nrm")
                nc.vector.tensor_tensor(nrm[:], qkv16, rscx[:], ALU.mult)
                # rope: out = nrm*cos2 + rot(nrm)*sin2, both tiles batched
                nrm4 = nrm[:].rearrange("p (u h) d -> p u h d", u=2)
                rt = bpool.tile([128, 2, 2 * HL, DH], dt.float16, tag="rt")
                nc.vector.tensor_tensor(rt[:], nrm4, cosb2, ALU.mult)
                rot = nrm4.rearrange("p u h (v d) -> p u h v d", v=2)[:, :, :, ::-1, :]
                rt2 = bpool.tile([128, 2, 2 * HL, DH], dt.float16, tag="rt2")
                nc.gpsimd.tensor_tensor(
                    rt2[:].rearrange("p u h (v d) -> p u h v d", v=2), rot,
                    sinb2.rearrange("p u h (v d) -> p u h v d", v=2), ALU.mult)
                nc.vector.tensor_add(qkn[:, 2 * sp:2 * sp + 2], rt[:], rt2[:])

                # transposes into [dh, seq] layout (2 heads per transpose)
                for u in range(2):
                    st = 2 * sp + u
                    tp_ps = psB.tile([128, 512], dt.float16, tag="sm")
                    qnr = qkn[:, st, 0:HL].rearrange("p h d -> p (h d)")
                    knr = qkn[:, st, HL:2 * HL].rearrange("p h d -> p (h d)")
                    nc.tensor.transpose(tp_ps[:, 0:128], qnr[:, 0:128], ident[:])
                    nc.tensor.transpose(tp_ps[:, 128:256], qnr[:, 128:256], ident[:])
                    nc.tensor.transpose(tp_ps[:, 256:384], knr[:, 0:128], ident[:])
                    nc.tensor.transpose(tp_ps[:, 384:512], knr[:, 128:256], ident[:])
                    nc.vector.tensor_copy(
                        qkT[:, :, st * 128:(st + 1) * 128],
                        tp_ps[:].rearrange("p (i s) -> p i s", i=4))

            # ---- sq = elu(qT)+1 = min(exp(qT), max(qT+1, 1)), fp16 ----
            qTv = qkT[:, 0:2]
            el1 = bpool.tile([128, 2, SEG], dt.float16, tag="el1")
            nc.scalar.activation(el1[:], qTv, AF.Exp)
            lin1 = bpool.tile([128, 2, SEG], dt.float16, tag="lin1")
            nc.vector.tensor_scalar(lin1[:], qTv, 1.0, 1.0, ALU.add, ALU.max)
            nc.vector.tensor_tensor(sqT[:], el1[:], lin1[:], ALU.min)

            # sk = elu(kn)+1, duplicated columns for the M update
            knv = qkn[:, :, HL:2 * HL, :]
            ek = bpool.tile([128, ST, HL, DH], dt.float16, tag="el1")
            nc.scalar.activation(ek[:], knv, AF.Exp)
            lin2 = bpool.tile([128, ST, HL, DH], dt.float16, tag="lin1")
            nc.vector.tensor_scalar(lin2[:], knv, 1.0, 1.0, ALU.add, ALU.max)
            sk0 = sk_all[:, :, :, 0, :]
            nc.vector.tensor_tensor(sk0, ek[:], lin2[:], ALU.min)
            nc.gpsimd.tensor_copy(sk_all[:, :, :, 1, :], sk0)

            # ---- phase 2: per head-pair attention ----
            for hp in range(2):
                pi = hp
                he, ho = 2 * hp, 2 * hp + 1
                P16e = ppool.tile([128, KC, SEG], dt.float16, tag="P16e")
                P16o = ppool.tile([128, KC, SEG], dt.float16, tag="P16o")
                P16s = {he: P16e, ho: P16o}

                # scores -> P = exp(S/8) in fp16, [k, q] layout; the two heads
                # of the pair use disjoint 64-row PE strips -> interleave them
                for qc in range(4):
                    ktn = 2 * qc + 2
                    for kt0 in range(0, ktn, 4):
                        nkt = min(4, ktn - kt0)
                        wv_e = psA.tile([128, nkt, 256], dt.float32, tag="big")
                        wv_o = psA.tile([128, nkt, 256], dt.float32, tag="big")
                        wvs = {he: wv_e, ho: wv_o}
                        for i in range(nkt):
                            kt = kt0 + i
                            for h2, po in ((he, 0), (ho, 64)):
                                nc.tensor.matmul(
                                    wvs[h2][:, i, :],
                                    lhsT=qkT[po:po + 64, 2 + pi, kt * 128:(kt + 1) * 128],
                                    rhs=qkT[po:po + 64, pi, qc * 256:(qc + 1) * 256],
                                    start=True, stop=True)
                        for h2 in (he, ho):
                            nc.scalar.activation(
                                P16s[h2][:, kt0:kt0 + nkt, qc * 256:(qc + 1) * 256],
                                wvs[h2][:], AF.Exp, scale=0.125)
                # all 8 diagonal blocks (2 per qc) in one op per head via a
                # hand-built [[1152, 8], [1, 128]] strided view of P16
                for h2 in (he, ho):
                    dgv = P16s[h2][:].rearrange("p a b -> p (a b)")[
                        :, 0:7 * 1152 + 1:1152][:, :, None].copy()
                    dgv.ap[2] = [1, 128]
                    nc.gpsimd.tensor_tensor(
                        dgv, dgv,
                        tri01[:, None, :].to_broadcast((128, 8, 128)),
                        ALU.mult)

                # AV + memory retrieval + combine, per head, in groups of 4
                for h, po in ((he, 0), (ho, 64)):
                    P16 = P16s[h]
                    for g2 in range(2):
                        av_ps = psB.tile([128, 4, DH + 1], dt.float32, tag="sm")
                        if seg > 0:
                            mem_ps = psB.tile([128, 4, DH + 1], dt.float32, tag="sm")
                        for qi in range(4):
                            qt = 4 * g2 + qi
                            for kt in range(qt + 1):
                                nc.tensor.matmul(
                                    av_ps[:, qi, :],
                                    lhsT=P16[:, kt, qt * 128:(qt + 1) * 128],
                                    rhs=v16[:, kt, h, 1:DH + 2],
                                    start=(kt == 0), stop=(kt == qt))
                            if seg > 0:
                                nc.tensor.matmul(
                                    mem_ps[:, qi, :],
                                    lhsT=sqT[po:po + 64, pi, qt * 128:(qt + 1) * 128],
                                    rhs=M16g[po:po + 64, h, :],
                                    start=True, stop=True)
                        # combine into staging; av col DH already = sum(P)/(1-g)
                        rl = tpool.tile([128, 4], dt.float32, tag="rl")
                        nc.vector.reciprocal(rl[:], av_ps[:, :, DH])
                        loc = outst[:, 4 * g2:4 * g2 + 4, h, :]
                        nc.vector.tensor_tensor(
                            loc, av_ps[:, :, 0:DH],
                            rl[:, :, None].to_broadcast((128, 4, DH)), ALU.mult)
                        if seg > 0:
                            # mem col 0 = sq.z (denominator); cols 1.. = g*sq.M
                            rm = tpool.tile([128, 4], dt.float32, tag="rm")
                            nc.vector.reciprocal(rm[:], mem_ps[:, :, 0])
                            cmb = tpool.tile([128, 4, DH], dt.float16, tag="cmb")
                            nc.vector.tensor_tensor(
                                cmb[:], mem_ps[:, :, 1:DH + 1],
                                rm[:, :, None].to_broadcast((128, 4, DH)), ALU.mult)
                            nc.vector.tensor_add(loc, loc, cmb[:])

                    # memory update (after retrieval reads of this segment)
                    mu_ps = psB.tile([128, DH + 1], dt.float32, tag="sm")
                    for st2 in range(ST):
                        nc.tensor.matmul(
                            mu_ps[:],
                            lhsT=sk_all[:, st2, h, :, :].rearrange("p u d -> p (u d)"),
                            rhs=v16[:, st2, h, 0:DH + 1],
                            start=(st2 == 0), stop=(st2 == ST - 1))
                    nc.vector.tensor_add(M_sb[:, h, :], M_sb[:, h, :], mu_ps[:])
                    # M16g: value part pre-scaled by g, z column (col 0) unscaled
                    nc.vector.tensor_scalar_mul(
                        M16g[:, h, 1:DH + 1], M_sb[:, h, 1:DH + 1], g_sb[:, h:h + 1])
                    nc.vector.tensor_copy(M16g[:, h, 0:1], M_sb[:, h, 0:1])

                # ---- output DMA for this head pair ----
                out_r = oa_r if seg < 2 else ox_r
                t0 = (seg % 2) * ST
                nc.sync.dma_start(
                    out_r[:, t0:t0 + ST, hp * 128:(hp + 1) * 128],
                    outst[:, :, 2 * hp:2 * hp + 2, :].rearrange(
                        "p t h d -> p t (h d)"))

    nc.compile()
    return nc


def prep_core_inputs(x, a, W_qkv_x, W_qkv_a, g_qx, g_kx, g_qa, g_ka, gate):
    """Host-side sharding: returns list of per-core input dicts."""
    x = np.asarray(x, np.float32)
    a = np.asarray(a, np.float32)
    W_qkv_x = np.asarray(W_qkv_x, np.float32)
    W_qkv_a = np.asarray(W_qkv_a, np.float32)
    gate = np.asarray(gate, np.float32)
    for gm in (g_qx, g_kx, g_qa, g_ka):
        assert np.allclose(np.asarray(gm), 1.0), "non-unit gamma not supported"

    # rope tables (global positions over packed [a; x])
    pos = np.arange(NTOT, dtype=np.float64)
    inv_freq = 1.0 / (10000.0 ** (np.arange(0, DH, 2, dtype=np.float64) / DH))
    ang = pos[:, None] * inv_freq[None, :]
    c, s = np.cos(ang), np.sin(ang)
    cos2 = np.concatenate([c, c], axis=1).astype(np.float16)      # [NTOT, 64]
    sin2 = np.concatenate([-s, s], axis=1).astype(np.float16)
    cos_t = np.ascontiguousarray(cos2.reshape(NSEG * ST, 128, DH).transpose(1, 0, 2))
    sin_t = np.ascontiguousarray(sin2.reshape(NSEG * ST, 128, DH).transpose(1, 0, 2))

    def wslice(W, heads):
        qs = [W[:, 64 * h + _PERM] for h in heads]
        ks = [W[:, DIM + 64 * h + _PERM] for h in heads]
        vs = [W[:, 2 * DIM + 64 * h:2 * DIM + 64 * h + 64] for h in heads]
        return np.ascontiguousarray(
            np.concatenate(qs + ks + vs, axis=1).astype(np.float16))

    gsig64 = 1.0 / (1.0 + np.exp(-gate.astype(np.float64)))
    gsig = gsig64.astype(np.float32)
    iomg = (1.0 / (1.0 - gsig64)).astype(np.float32)

    at_b = [np.ascontiguousarray(a[b].T.astype(np.float16)) for b in range(B)]
    xt_b = [np.ascontiguousarray(x[b].T.astype(np.float16)) for b in range(B)]

    ins = []
    for c_ in range(N_CORES):
        b, hg = divmod(c_, 4)
        heads = [4 * hg + i for i in range(HL)]
        ins.append({
            "at": at_b[b],
            "xt": xt_b[b],
            "wa": wslice(W_qkv_a, heads),
            "wx": wslice(W_qkv_x, heads),
            "cos2": cos_t,
            "sin2": sin_t,
            "gvec": np.ascontiguousarray(
                np.stack([gsig[heads], iomg[heads]], axis=0)),
        })
    return ins


def assemble_outputs(results):
    out_x = np.empty((B, NX, DIM), np.float32)
    out_a = np.empty((B, NA, DIM), np.float32)
    for c_ in range(N_CORES):
        b, hg = divmod(c_, 4)
        out_x[b, :, 256 * hg:256 * (hg + 1)] = results[c_]["out_x"].astype(np.float32)
        out_a[b, :, 256 * hg:256 * (hg + 1)] = results[c_]["out_a"].astype(np.float32)
    return out_x, out_a


_PROGRAM_CACHE = {}


def get_program():
    if "nc" not in _PROGRAM_CACHE:
        _PROGRAM_CACHE["nc"] = build_program()
    return _PROGRAM_CACHE["nc"]


def kernel(**inputs):
    from concourse import bass_utils
    nc = get_program()
    ins = prep_core_inputs(**inputs)
    res = bass_utils.run_bass_kernel_spmd(nc, ins, core_ids=list(range(N_CORES)))
    return assemble_outputs(res.results)
